# revision 23
# baseline (speedup 1.0000x reference)
"""MultiHeadLatentAttention TRN2 kernel.

Sharding: 8 cores = 2 batches x 4 head-groups (4 heads of 128 dims each).
Each core computes, for its (batch, 4 heads):
    latS = Wdkv^T xTs         [256, 512]  (this core's S-block only; the full
                                           latT is AllGathered across the
                                           batch's 4 cores on the DMA/CCE path
                                           while the PE runs the q-projection)
    qT_h = Wq_h^T xT          [hd, S]     (fp16 matmuls, fp32 psum)
    kT_h = Wuk_h^T latT       [hd, S]     (S-block sb computed just-in-time,
    v_h  = latT^T Wuv_h       [S, hd]      interleaved into attention bubbles)
    scoresT = k qT            [keys, q]   (transposed scores: no transposes)
    expT = exp(scale*scoresT) (causal: skip invalid blocks, tri-mask diagonal,
                               gpsimd zero-fills the masked strip)
    den  = ones^T exsum       [128, q]    (pair-sums accumulated on DVE; one
                                           all-ones matmul per (h, qb))
    ctxT = v^T expT / den     [hd, q]
    part = sum_h ctxT_h^T Wout_h  [S, dout]  (row-parallel out-proj partial, bf16)
Host sums the 4 partials per batch and adds b_out.

All inputs are pre-swizzled on the host into [128, X] row-contiguous DRAM
layouts so every load is descriptor-minimal, split into pieces for arrival
pacing across the sync/scalar HWDGE rings. The attention inner loop is
software-pipelined (scores of pair p+1 issue before ctx of pair p), and the
PE bubbles left by the exp chain are filled with deferred work units:
out-proj chunks of q-block qb-1 and kT/vT S-blocks needed at qb+1.
"""

import sys

_BASS_REPO = "/opt/trn_rl_repo"
if _BASS_REPO not in sys.path:
    sys.path.insert(0, _BASS_REPO)

import numpy as np

import concourse.bass as bass  # noqa: F401
import concourse.mybir as mybir
import concourse.tile as tile
from concourse import bacc, bass_utils

F32 = mybir.dt.float32
F16 = mybir.dt.float16
BF16 = mybir.dt.bfloat16

B = 2
S = 2048
DIN = 2048
DOUT = 2048
NH = 16
HD = 128
LAT = 256
NCORES = 8
HEADS_PER_CORE = 4
COLS_PER_CORE = HEADS_PER_CORE * HD  # 512

KC = DIN // 128  # 16 contraction chunks over d_in
NB = S // 512    # 4 blocks of 512 over S
NT = S // 128    # 16 tiles of 128 over S
SCALE = 1.0 / float(np.sqrt(HD))

_CACHE = {}


def _build():
    nc = bacc.Bacc("TRN2", target_bir_lowering=False, debug=False,
                   num_devices=NCORES)

    # pre-swizzled [128, X] inputs (chunk-major along the free axis)
    xt_d = nc.dram_tensor("xt", [128, KC * S], F16, kind="ExternalInput")
    wq_d = nc.dram_tensor("wq", [128, KC * COLS_PER_CORE], F16,
                          kind="ExternalInput")
    wdkv_d = nc.dram_tensor("wdkv", [128, KC * LAT], F16, kind="ExternalInput")
    wuk_d = nc.dram_tensor("wuk", [128, 2 * COLS_PER_CORE], F16,
                           kind="ExternalInput")
    wuv_d = nc.dram_tensor("wuv", [128, 2 * COLS_PER_CORE], F16,
                           kind="ExternalInput")
    wout_d = nc.dram_tensor("wout", [128, 4 * DOUT], F16, kind="ExternalInput")
    mask_d = nc.dram_tensor("mask", [128, 128], F16, kind="ExternalInput")
    out_d = nc.dram_tensor("out", [S, DOUT], BF16, kind="ExternalOutput")

    Exp = mybir.ActivationFunctionType.Exp

    with tile.TileContext(nc) as tc:
        with (
            tc.tile_pool(name="consts", bufs=1) as cpool,
            tc.tile_pool(name="persist", bufs=1) as ppool,
            tc.tile_pool(name="acts", bufs=1) as apool,
        ):
            # ---- constants ----
            ones_t = cpool.tile([128, 128], F16, name="ones_t", tag="ones_t")
            nc.vector.memset(ones_t[:], 1.0)
            mask_t = cpool.tile([128, 128], F16, name="mask_t", tag="mask_t")

            # ---- persistent weights (used by deferred phase-2 units) ----
            wout_t = ppool.tile([128, HEADS_PER_CORE, DOUT], F16,
                                name="wout_t", tag="wout_t")
            wuk_t = ppool.tile([128, 2, COLS_PER_CORE], F16, name="wuk_t",
                               tag="wuk_t")
            wuv_t = ppool.tile([128, 2, COLS_PER_CORE], F16, name="wuv_t",
                               tag="wuv_t")

            # ---- persistent activations ----
            latT = [apool.tile([128, S], F16, name=f"latT{m}", tag=f"latT{m}")
                    for m in range(LAT // 128)]
            qT = [apool.tile([128, S], F16, name=f"qT{h}", tag=f"qT{h}")
                  for h in range(HEADS_PER_CORE)]
            kT = [apool.tile([128, S], F16, name=f"kT{h}", tag=f"kT{h}")
                  for h in range(HEADS_PER_CORE)]
            vt = [apool.tile([128, S], F16, name=f"vt{h}", tag=f"vt{h}")
                  for h in range(HEADS_PER_CORE)]
            ctxT = [apool.tile([128, S], F16, name=f"ctxT{h}", tag=f"ctxT{h}")
                    for h in range(HEADS_PER_CORE)]

            # ============= phase 1: projections =============
            with tc.tile_pool(name="p1w", bufs=1) as p1pool:
                xt_t = p1pool.tile([128, KC, S], F16, name="xt_t", tag="xt_t")
                wdkv_t = p1pool.tile([128, KC, LAT], F16, name="wdkv_t",
                                     tag="wdkv_t")
                wq_t = p1pool.tile([128, KC, COLS_PER_CORE], F16, name="wq_t",
                                   tag="wq_t")

                xt_src = xt_d.ap().rearrange("p (k s) -> p k s", s=S)
                wdkv_src = wdkv_d.ap().rearrange("p (k s) -> p k s", s=LAT)
                # the latent projection alone consumes xt at ~350 GB/s, so xt
                # pieces alternate across BOTH HWDGE rings, with wdkv pieces
                # woven in chunk-aligned; the remaining weights follow on the
                # scalar ring (needed much later).
                nc.scalar.dma_start(wdkv_t[:, 0:2, :], wdkv_src[:, 0:2, :])
                nc.sync.dma_start(xt_t[:, 0:1, :], xt_src[:, 0:1, :])
                nc.scalar.dma_start(xt_t[:, 1:2, :], xt_src[:, 1:2, :])
                nc.sync.dma_start(xt_t[:, 2:5, :], xt_src[:, 2:5, :])
                nc.scalar.dma_start(wdkv_t[:, 2:6, :], wdkv_src[:, 2:6, :])
                nc.scalar.dma_start(xt_t[:, 5:8, :], xt_src[:, 5:8, :])
                nc.sync.dma_start(xt_t[:, 8:11, :], xt_src[:, 8:11, :])
                nc.scalar.dma_start(wdkv_t[:, 6:KC, :], wdkv_src[:, 6:KC, :])
                nc.scalar.dma_start(xt_t[:, 11:14, :], xt_src[:, 11:14, :])
                nc.sync.dma_start(xt_t[:, 14:KC, :], xt_src[:, 14:KC, :])
                nc.scalar.dma_start(wq_t[:],
                                    wq_d.ap().rearrange("p (k s) -> p k s",
                                                        s=COLS_PER_CORE))
                nc.scalar.dma_start(wuk_t[:],
                                    wuk_d.ap().rearrange("p (k s) -> p k s",
                                                         s=COLS_PER_CORE))
                nc.scalar.dma_start(wuv_t[:],
                                    wuv_d.ap().rearrange("p (k s) -> p k s",
                                                         s=COLS_PER_CORE))
                nc.scalar.dma_start(wout_t[:],
                                    wout_d.ap().rearrange("p (k s) -> p k s",
                                                          s=DOUT))
                nc.scalar.dma_start(mask_t[:], mask_d.ap())

                with tc.tile_pool(name="pproj", bufs=8, space="PSUM") as pproj:
                    # brief PE warmup while the first DMA pieces land
                    warm = pproj.tile([128, 512], F32, name="warm", tag="pp")
                    for _ in range(16):
                        nc.tensor.matmul(warm[:, 0:128], ones_t[:],
                                         ones_t[:], start=True, stop=True)

                    def kmajor(groups, lhs_of, rhs_of, nk, out_of,
                               copy_eng=None):
                        """Accumulate len(groups) psum banks over nk chunks,
                        chunk-major so compute starts on the first DMA."""
                        pls = [pproj.tile([128, 512], F32, name=f"pp{i}",
                                          tag="pp")
                               for i in range(len(groups))]
                        for k in range(nk):
                            for i, g in enumerate(groups):
                                nc.tensor.matmul(pls[i][:], lhs_of(k, g),
                                                 rhs_of(k, g),
                                                 start=(k == 0),
                                                 stop=(k == nk - 1))
                        for i, g in enumerate(groups):
                            eng = copy_eng(i) if copy_eng else "scalar"
                            if eng == "scalar":
                                nc.scalar.copy(out_of(g), pls[i][:])
                            else:
                                nc.vector.tensor_copy(out_of(g), pls[i][:])

                    sb_sl = lambda sb: slice(512 * sb, 512 * (sb + 1))

                    # latT = Wdkv^T xT   (8 groups: 2 m x 4 sb)
                    kmajor(
                        [(m, sb) for m in range(2) for sb in range(NB)],
                        lambda k, g: wdkv_t[:, k, 128 * g[0]:128 * (g[0] + 1)],
                        lambda k, g: xt_t[:, k, sb_sl(g[1])],
                        KC,
                        lambda g: latT[g[0]][:, sb_sl(g[1])])

                    # qT_h = Wq_h^T xT   (two batches of 8 groups: 2 h x 4 sb)
                    for h0 in (0, 2):
                        kmajor(
                            [(h0 + dh, sb) for dh in range(2)
                             for sb in range(NB)],
                            lambda k, g: wq_t[:, k, 128 * g[0]:128 * (g[0] + 1)],
                            lambda k, g: xt_t[:, k, sb_sl(g[1])],
                            KC,
                            lambda g: qT[g[0]][:, sb_sl(g[1])])

                    # kT / vT for S-block 0 only; later S-blocks are deferred
                    # into the attention phase (block sb is first read at
                    # q-block sb)
                    kmajor(
                        [(h, 0) for h in range(HEADS_PER_CORE)],
                        lambda k, g: wuk_t[:, k, 128 * g[0]:128 * (g[0] + 1)],
                        lambda k, g: latT[k][:, sb_sl(g[1])],
                        2,
                        lambda g: kT[g[0]][:, sb_sl(g[1])],
                        copy_eng=lambda i: "vector")
                    for h in range(HEADS_PER_CORE):
                        pv = pproj.tile([128, 512], F32, name="pv", tag="pp")
                        for j in range(4):
                            for m in range(LAT // 128):
                                nc.tensor.matmul(
                                    pv[:, 128 * j:128 * (j + 1)],
                                    latT[m][:, 128 * j:128 * (j + 1)],
                                    wuv_t[:, m, 128 * h:128 * (h + 1)],
                                    start=(m == 0),
                                    stop=(m == LAT // 128 - 1))
                        if h % 2 == 0:
                            nc.vector.tensor_copy(vt[h][:, 0:512], pv[:])
                        else:
                            nc.scalar.copy(vt[h][:, 0:512], pv[:])

            # ========= phase 2: attention + interleaved deferred work =========
            # key tiles processed in pairs -> one [128,1024] exp per pair.
            # software pipeline: scores of pair p+1 issue before ctx of pair
            # p; one deferred unit (out-proj chunk of qb-1, or a kT/vT
            # S-block for qb+1) is emitted per pair to fill the exp latency.
            with (
                tc.tile_pool(name="temps", bufs=1) as tpool,
                tc.tile_pool(name="psc", bufs=2, space="PSUM") as psc,
                tc.tile_pool(name="pctx", bufs=2, space="PSUM") as pctx,
                tc.tile_pool(name="pden", bufs=2, space="PSUM") as pden,
            ):
                osb_live = {}

                def po_unit(stt, ob):
                    if ob == 0:
                        osb_live[stt] = tpool.tile([128, DOUT], BF16,
                                                   name="osb", tag="osb",
                                                   bufs=2)
                    osb = osb_live[stt]
                    po = pden.tile([128, 512], F32, name="po", tag="den")
                    for h in range(HEADS_PER_CORE):
                        nc.tensor.matmul(
                            po[:],
                            ctxT[h][:, 128 * stt:128 * (stt + 1)],
                            wout_t[:, h, 512 * ob:512 * (ob + 1)],
                            start=(h == 0), stop=(h == HEADS_PER_CORE - 1))
                    dst = osb[:, 512 * ob:512 * (ob + 1)]
                    if ob % 2 == 0:
                        nc.vector.tensor_copy(dst, po[:])
                    else:
                        nc.scalar.copy(dst, po[:])
                    if ob == NB - 1:
                        nc.sync.dma_start(
                            out_d.ap()[128 * stt:128 * (stt + 1), :], osb[:])
                        del osb_live[stt]

                def kt_unit(h, sb):
                    pk = pden.tile([128, 512], F32, name="pk", tag="den")
                    for k in range(2):
                        nc.tensor.matmul(pk[:],
                                         wuk_t[:, k, 128 * h:128 * (h + 1)],
                                         latT[k][:, 512 * sb:512 * (sb + 1)],
                                         start=(k == 0), stop=(k == 1))
                    nc.vector.tensor_copy(kT[h][:, 512 * sb:512 * (sb + 1)],
                                          pk[:])

                def vt_unit(h, st4):
                    pv = pden.tile([128, 512], F32, name="pv2", tag="den")
                    for j in range(4):
                        stt = 4 * st4 + j
                        for m in range(LAT // 128):
                            nc.tensor.matmul(
                                pv[:, 128 * j:128 * (j + 1)],
                                latT[m][:, 128 * stt:128 * (stt + 1)],
                                wuv_t[:, m, 128 * h:128 * (h + 1)],
                                start=(m == 0), stop=(m == LAT // 128 - 1))
                    nc.vector.tensor_copy(vt[h][:, 512 * st4:512 * (st4 + 1)],
                                          pv[:])

                for qb in range(NB):
                    units = []
                    if qb + 1 < NB:
                        for h in range(HEADS_PER_CORE):
                            units.append((kt_unit, (h, qb + 1)))
                            units.append((vt_unit, (h, qb + 1)))
                    if qb > 0:
                        for stt in range(4 * (qb - 1), 4 * qb):
                            for ob in range(NB):
                                units.append((po_unit, (stt, ob)))
                    units = iter(units)

                    for h in range(HEADS_PER_CORE):
                        ps_ctx = pctx.tile([128, 512], F32, name="ps_ctx",
                                           tag="ctx")
                        exsum = tpool.tile([128, 512], F16, name="exsum",
                                           tag="exsum", bufs=2)
                        nkt = 4 * qb + 4
                        pend = []  # software pipeline, depth 2

                        def finish(p):
                            kt0, col0, ps_sc_, ex_ = p
                            # pair-sum + running key-sum on DVE: a single
                            # all-ones denominator matmul per (h, qb)
                            if kt0 == 0:
                                nc.vector.tensor_add(exsum[:, 0:512],
                                                     ex_[:, 0:512],
                                                     ex_[:, 512:1024])
                            else:
                                exs = tpool.tile([128, 512], F16, name="exs",
                                                 tag="exs", bufs=3)
                                nc.vector.tensor_add(exs[:, col0:512],
                                                     ex_[:, col0:512],
                                                     ex_[:, 512 + col0:1024])
                                nc.vector.tensor_add(exsum[:, col0:512],
                                                     exsum[:, col0:512],
                                                     exs[:, col0:512])
                            for half in range(2):
                                kt = kt0 + half
                                nc.tensor.matmul(
                                    ps_ctx[:, col0:512],
                                    vt[h][:, 128 * kt:128 * (kt + 1)],
                                    ex_[:, 512 * half + col0:512 * (half + 1)],
                                    start=(kt0 == 0 and half == 0),
                                    stop=(kt == nkt - 1))

                        for kt0 in range(0, nkt, 2):
                            pair = (kt0, kt0 + 1)
                            # valid q start (block-local) per kt; pair shares
                            # the wider (earlier) start col0 of ktA
                            djA = pair[0] - 4 * qb
                            col0 = 128 * djA if djA >= 0 else 0
                            qhi = 512 * (qb + 1)
                            ps_sc = psc.tile([128, 1024], F32, name="ps_sc",
                                             tag="sc")
                            ex = tpool.tile([128, 1024], F16, name="ex",
                                            tag="ex", bufs=4)
                            for half, kt in enumerate(pair):
                                dj = kt - 4 * qb
                                c = 128 * dj if dj >= 0 else 0
                                nc.tensor.matmul(
                                    ps_sc[:, 512 * half + c:512 * (half + 1)],
                                    kT[h][:, 128 * kt:128 * (kt + 1)],
                                    qT[h][:, 512 * qb + c:qhi],
                                    start=True, stop=True,
                                    skip_group_check=True)
                            # one wide exp for the pair (psum -> sbuf fp16);
                            # the half1 strip [512+col0, 512+col0+128) sees
                            # stale psum (bounded), gpsimd zeroes it below.
                            nc.scalar.activation(ex[:, col0:1024],
                                                 ps_sc[:, col0:1024], Exp,
                                                 scale=SCALE)
                            if djA >= 0:
                                nc.gpsimd.memset(
                                    ex[:, 512 + col0:512 + col0 + 128], 0.0)
                            for half, kt in enumerate(pair):
                                dj = kt - 4 * qb
                                if dj >= 0:
                                    c = 128 * dj
                                    nc.gpsimd.tensor_mul(
                                        ex[:, 512 * half + c:512 * half + c + 128],
                                        ex[:, 512 * half + c:512 * half + c + 128],
                                        mask_t[:])
                            # one deferred unit fills the PE while this
                            # pair's exp/mask chain completes
                            u = next(units, None)
                            if u is not None:
                                u[0](*u[1])
                            pend.append((kt0, col0, ps_sc, ex))
                            if len(pend) > 2:
                                finish(pend.pop(0))
                        for p in pend:
                            finish(p)

                        ps_den = pden.tile([128, 512], F32, name="ps_den",
                                           tag="den")
                        nc.tensor.matmul(ps_den[:], ones_t[:], exsum[:],
                                         start=True, stop=True)
                        rden = tpool.tile([128, 512], F32, name="rden",
                                          tag="rden", bufs=2)
                        nc.vector.reciprocal_approx_fast(rden[:], ps_den[:])
                        nc.vector.tensor_mul(ctxT[h][:, 512 * qb:512 * (qb + 1)],
                                             ps_ctx[:], rden[:])

                    # any leftover deferred units, and qb3's own out-proj
                    for fn, args in units:
                        fn(*args)
                    if qb == NB - 1:
                        for stt in range(4 * qb, 4 * qb + 4):
                            for ob in range(NB):
                                po_unit(stt, ob)

    nc.compile()
    return nc


def _get_nc():
    if "nc" not in _CACHE:
        _CACHE["nc"] = _build()
    return _CACHE["nc"]


def _swz(a, chunk_rows=128):
    """[C*128, F] -> [128, C*F] chunk-major row-contiguous swizzle."""
    cr, f = a.shape
    c = cr // chunk_rows
    return np.ascontiguousarray(
        a.reshape(c, chunk_rows, f).transpose(1, 0, 2).reshape(chunk_rows,
                                                               c * f))


def _make_in_maps(x, W_query, W_DKV, W_UK, W_UV, W_out):
    mask = np.triu(np.ones((128, 128), dtype=np.float16))
    wdkv16 = _swz(W_DKV.astype(np.float16))
    xT16 = [_swz(x[b].T.astype(np.float16)) for b in range(B)]
    in_maps = []
    for c in range(NCORES):
        b = c // 4
        g = c % 4
        cols = slice(512 * g, 512 * (g + 1))
        in_maps.append({
            "xt": xT16[b],
            "wq": _swz(W_query[:, cols].astype(np.float16)),
            "wdkv": wdkv16,
            "wuk": _swz(W_UK[:, cols].astype(np.float16)),
            "wuv": _swz(W_UV[:, cols].astype(np.float16)),
            "wout": _swz(W_out[cols, :].astype(np.float16)),
            "mask": mask,
        })
    return in_maps


def run_on_device(x, W_query, W_DKV, W_UK, W_UV, W_out, **run_kwargs):
    nc = _get_nc()
    in_maps = _make_in_maps(x, W_query, W_DKV, W_UK, W_UV, W_out)
    return bass_utils.run_bass_kernel_spmd(
        nc, in_maps, core_ids=list(range(NCORES)), **run_kwargs)


def kernel(x, W_query, W_DKV, W_UK, W_UV, W_out, b_out):
    x = np.asarray(x, dtype=np.float32)
    W_query = np.asarray(W_query, dtype=np.float32)
    W_DKV = np.asarray(W_DKV, dtype=np.float32)
    W_UK = np.asarray(W_UK, dtype=np.float32)
    W_UV = np.asarray(W_UV, dtype=np.float32)
    W_out = np.asarray(W_out, dtype=np.float32)
    b_out = np.asarray(b_out, dtype=np.float32)

    res = None
    for attempt in range(3):
        try:
            res = run_on_device(x, W_query, W_DKV, W_UK, W_UV, W_out)
            break
        except Exception:
            if attempt == 2:
                raise
    out = np.empty((B, S, DOUT), dtype=np.float32)
    for b in range(B):
        acc = np.asarray(res.results[4 * b]["out"], dtype=np.float32)
        for g in range(1, 4):
            acc += np.asarray(res.results[4 * b + g]["out"], dtype=np.float32)
        out[b] = acc + b_out[None, :]
    return out


# revision 25
# speedup vs baseline: 1.0477x; 1.0477x over previous
"""MultiHeadLatentAttention TRN2 kernel.

Sharding: 8 cores = 2 batches x 4 head-groups (4 heads of 128 dims each).
Each core computes, for its (batch, 4 heads):
    latS = Wdkv^T xTs         [256, 512]  (this core's S-block only; the full
                                           latT is AllGathered across the
                                           batch's 4 cores on the DMA/CCE path
                                           while the PE runs the q-projection)
    qT_h = Wq_h^T xT          [hd, S]     (fp16 matmuls, fp32 psum)
    kT_h = Wuk_h^T latT       [hd, S]     (S-block sb computed just-in-time,
    v_h  = latT^T Wuv_h       [S, hd]      interleaved into attention bubbles)
    scoresT = k qT            [keys, q]   (transposed scores: no transposes)
    expT = exp(scale*scoresT) (causal: skip invalid blocks, tri-mask diagonal,
                               gpsimd zero-fills the masked strip)
    den  = ones^T exsum       [128, q]    (pair-sums accumulated on DVE; one
                                           all-ones matmul per (h, qb))
    ctxT = v^T expT / den     [hd, q]
    part = sum_h ctxT_h^T Wout_h  [S, dout]  (row-parallel out-proj partial, bf16)
Host sums the 4 partials per batch and adds b_out.

All inputs are pre-swizzled on the host into [128, X] row-contiguous DRAM
layouts so every load is descriptor-minimal, split into pieces for arrival
pacing across the sync/scalar HWDGE rings. The attention inner loop is
software-pipelined (scores of pair p+1 issue before ctx of pair p), and the
PE bubbles left by the exp chain are filled with deferred work units:
out-proj chunks of q-block qb-1 and kT/vT S-blocks needed at qb+1.
"""

import sys

_BASS_REPO = "/opt/trn_rl_repo"
if _BASS_REPO not in sys.path:
    sys.path.insert(0, _BASS_REPO)

import numpy as np

import concourse.bass as bass  # noqa: F401
import concourse.mybir as mybir
import concourse.tile as tile
from concourse import bacc, bass_utils

F32 = mybir.dt.float32
F16 = mybir.dt.float16
BF16 = mybir.dt.bfloat16

B = 2
S = 2048
DIN = 2048
DOUT = 2048
NH = 16
HD = 128
LAT = 256
NCORES = 8
HEADS_PER_CORE = 4
COLS_PER_CORE = HEADS_PER_CORE * HD  # 512

KC = DIN // 128  # 16 contraction chunks over d_in
NB = S // 512    # 4 blocks of 512 over S
NT = S // 128    # 16 tiles of 128 over S
SCALE = 1.0 / float(np.sqrt(HD))

_CACHE = {}


def _build():
    nc = bacc.Bacc("TRN2", target_bir_lowering=False, debug=False,
                   num_devices=NCORES)

    # pre-swizzled [128, X] inputs (chunk-major along the free axis)
    xt_d = nc.dram_tensor("xt", [128, KC * S], F16, kind="ExternalInput")
    wq_d = nc.dram_tensor("wq", [128, KC * COLS_PER_CORE], F16,
                          kind="ExternalInput")
    wdkv_d = nc.dram_tensor("wdkv", [128, KC * LAT], F16, kind="ExternalInput")
    wuk_d = nc.dram_tensor("wuk", [128, 2 * COLS_PER_CORE], F16,
                           kind="ExternalInput")
    wuv_d = nc.dram_tensor("wuv", [128, 2 * COLS_PER_CORE], F16,
                           kind="ExternalInput")
    wout_d = nc.dram_tensor("wout", [128, 4 * DOUT], F16, kind="ExternalInput")
    mask_d = nc.dram_tensor("mask", [128, 128], F16, kind="ExternalInput")
    out_d = nc.dram_tensor("out", [S, DOUT], BF16, kind="ExternalOutput")

    Exp = mybir.ActivationFunctionType.Exp

    with tile.TileContext(nc) as tc:
        with (
            tc.tile_pool(name="consts", bufs=1) as cpool,
            tc.tile_pool(name="persist", bufs=1) as ppool,
            tc.tile_pool(name="acts", bufs=1) as apool,
        ):
            # ---- constants ----
            ones_t = cpool.tile([128, 128], F16, name="ones_t", tag="ones_t")
            nc.vector.memset(ones_t[:], 1.0)
            mask_t = cpool.tile([128, 128], F16, name="mask_t", tag="mask_t")

            # ---- persistent weights (used by deferred phase-2 units) ----
            wout_t = ppool.tile([128, HEADS_PER_CORE, DOUT], F16,
                                name="wout_t", tag="wout_t")
            wuk_t = ppool.tile([128, 2, COLS_PER_CORE], F16, name="wuk_t",
                               tag="wuk_t")
            wuv_t = ppool.tile([128, 2, COLS_PER_CORE], F16, name="wuv_t",
                               tag="wuv_t")

            # ---- persistent activations ----
            latT = [apool.tile([128, S], F16, name=f"latT{m}", tag=f"latT{m}")
                    for m in range(LAT // 128)]
            qT = [apool.tile([128, S], F16, name=f"qT{h}", tag=f"qT{h}")
                  for h in range(HEADS_PER_CORE)]
            kT = [apool.tile([128, S], F16, name=f"kT{h}", tag=f"kT{h}")
                  for h in range(HEADS_PER_CORE)]
            vt = [apool.tile([128, S], F16, name=f"vt{h}", tag=f"vt{h}")
                  for h in range(HEADS_PER_CORE)]
            ctxT = [apool.tile([128, S], F16, name=f"ctxT{h}", tag=f"ctxT{h}")
                    for h in range(HEADS_PER_CORE)]

            # ============= phase 1: projections =============
            with tc.tile_pool(name="p1w", bufs=1) as p1pool:
                xt_t = p1pool.tile([128, KC, S], F16, name="xt_t", tag="xt_t")
                wdkv_t = p1pool.tile([128, KC, LAT], F16, name="wdkv_t",
                                     tag="wdkv_t")
                wq_t = p1pool.tile([128, KC, COLS_PER_CORE], F16, name="wq_t",
                                   tag="wq_t")

                xt_src = xt_d.ap().rearrange("p (k s) -> p k s", s=S)
                wdkv_src = wdkv_d.ap().rearrange("p (k s) -> p k s", s=LAT)
                # the latent projection alone consumes xt at ~350 GB/s, so xt
                # pieces alternate across BOTH HWDGE rings, with wdkv pieces
                # woven in chunk-aligned; the remaining weights follow on the
                # scalar ring (needed much later).
                nc.scalar.dma_start(wdkv_t[:, 0:2, :], wdkv_src[:, 0:2, :])
                nc.sync.dma_start(xt_t[:, 0:1, :], xt_src[:, 0:1, :])
                nc.scalar.dma_start(xt_t[:, 1:2, :], xt_src[:, 1:2, :])
                nc.sync.dma_start(xt_t[:, 2:4, :], xt_src[:, 2:4, :])
                nc.scalar.dma_start(wdkv_t[:, 2:8, :], wdkv_src[:, 2:8, :])
                nc.scalar.dma_start(xt_t[:, 4:6, :], xt_src[:, 4:6, :])
                nc.sync.dma_start(xt_t[:, 6:9, :], xt_src[:, 6:9, :])
                nc.scalar.dma_start(wdkv_t[:, 8:KC, :], wdkv_src[:, 8:KC, :])
                nc.scalar.dma_start(xt_t[:, 9:12, :], xt_src[:, 9:12, :])
                nc.sync.dma_start(xt_t[:, 12:KC, :], xt_src[:, 12:KC, :])
                nc.scalar.dma_start(wq_t[:],
                                    wq_d.ap().rearrange("p (k s) -> p k s",
                                                        s=COLS_PER_CORE))
                nc.scalar.dma_start(wuk_t[:],
                                    wuk_d.ap().rearrange("p (k s) -> p k s",
                                                         s=COLS_PER_CORE))
                nc.scalar.dma_start(wuv_t[:],
                                    wuv_d.ap().rearrange("p (k s) -> p k s",
                                                         s=COLS_PER_CORE))
                nc.scalar.dma_start(wout_t[:],
                                    wout_d.ap().rearrange("p (k s) -> p k s",
                                                          s=DOUT))
                nc.scalar.dma_start(mask_t[:], mask_d.ap())

                with tc.tile_pool(name="pproj", bufs=8, space="PSUM") as pproj:
                    # brief PE warmup while the first DMA pieces land
                    warm = pproj.tile([128, 512], F32, name="warm", tag="pp")
                    for _ in range(16):
                        nc.tensor.matmul(warm[:, 0:128], ones_t[:],
                                         ones_t[:], start=True, stop=True)

                    def kmajor(groups, lhs_of, rhs_of, nk, out_of,
                               copy_eng=None):
                        """Accumulate len(groups) psum banks over nk chunks,
                        chunk-major so compute starts on the first DMA."""
                        pls = [pproj.tile([128, 512], F32, name=f"pp{i}",
                                          tag="pp")
                               for i in range(len(groups))]
                        for k in range(nk):
                            for i, g in enumerate(groups):
                                nc.tensor.matmul(pls[i][:], lhs_of(k, g),
                                                 rhs_of(k, g),
                                                 start=(k == 0),
                                                 stop=(k == nk - 1))
                        for i, g in enumerate(groups):
                            eng = copy_eng(i) if copy_eng else "scalar"
                            if eng == "scalar":
                                nc.scalar.copy(out_of(g), pls[i][:])
                            else:
                                nc.vector.tensor_copy(out_of(g), pls[i][:])

                    sb_sl = lambda sb: slice(512 * sb, 512 * (sb + 1))

                    # latT = Wdkv^T xT   (8 groups: 2 m x 4 sb)
                    kmajor(
                        [(m, sb) for m in range(2) for sb in range(NB)],
                        lambda k, g: wdkv_t[:, k, 128 * g[0]:128 * (g[0] + 1)],
                        lambda k, g: xt_t[:, k, sb_sl(g[1])],
                        KC,
                        lambda g: latT[g[0]][:, sb_sl(g[1])])

                    # qT_h = Wq_h^T xT   (two batches of 8 groups: 2 h x 4 sb)
                    for h0 in (0, 2):
                        kmajor(
                            [(h0 + dh, sb) for dh in range(2)
                             for sb in range(NB)],
                            lambda k, g: wq_t[:, k, 128 * g[0]:128 * (g[0] + 1)],
                            lambda k, g: xt_t[:, k, sb_sl(g[1])],
                            KC,
                            lambda g: qT[g[0]][:, sb_sl(g[1])])

                    # kT / vT for S-block 0 only; later S-blocks are deferred
                    # into the attention phase (block sb is first read at
                    # q-block sb)
                    kmajor(
                        [(h, 0) for h in range(HEADS_PER_CORE)],
                        lambda k, g: wuk_t[:, k, 128 * g[0]:128 * (g[0] + 1)],
                        lambda k, g: latT[k][:, sb_sl(g[1])],
                        2,
                        lambda g: kT[g[0]][:, sb_sl(g[1])],
                        copy_eng=lambda i: "vector")
                    for h in range(HEADS_PER_CORE):
                        pv = pproj.tile([128, 512], F32, name="pv", tag="pp")
                        for j in range(4):
                            for m in range(LAT // 128):
                                nc.tensor.matmul(
                                    pv[:, 128 * j:128 * (j + 1)],
                                    latT[m][:, 128 * j:128 * (j + 1)],
                                    wuv_t[:, m, 128 * h:128 * (h + 1)],
                                    start=(m == 0),
                                    stop=(m == LAT // 128 - 1))
                        if h % 2 == 0:
                            nc.vector.tensor_copy(vt[h][:, 0:512], pv[:])
                        else:
                            nc.scalar.copy(vt[h][:, 0:512], pv[:])

            # ========= phase 2: attention + interleaved deferred work =========
            # key tiles processed in pairs -> one [128,1024] exp per pair.
            # software pipeline: scores of pair p+1 issue before ctx of pair
            # p; one deferred unit (out-proj chunk of qb-1, or a kT/vT
            # S-block for qb+1) is emitted per pair to fill the exp latency.
            with (
                tc.tile_pool(name="temps", bufs=1) as tpool,
                tc.tile_pool(name="psc", bufs=2, space="PSUM") as psc,
                tc.tile_pool(name="pctx", bufs=2, space="PSUM") as pctx,
                tc.tile_pool(name="pden", bufs=2, space="PSUM") as pden,
            ):
                osb_live = {}

                def po_unit(stt, ob):
                    if ob == 0:
                        osb_live[stt] = tpool.tile([128, DOUT], BF16,
                                                   name="osb", tag="osb",
                                                   bufs=2)
                    osb = osb_live[stt]
                    po = pden.tile([128, 512], F32, name="po", tag="den")
                    for h in range(HEADS_PER_CORE):
                        nc.tensor.matmul(
                            po[:],
                            ctxT[h][:, 128 * stt:128 * (stt + 1)],
                            wout_t[:, h, 512 * ob:512 * (ob + 1)],
                            start=(h == 0), stop=(h == HEADS_PER_CORE - 1))
                    dst = osb[:, 512 * ob:512 * (ob + 1)]
                    if ob % 2 == 0:
                        nc.vector.tensor_copy(dst, po[:])
                    else:
                        nc.scalar.copy(dst, po[:])
                    if ob == NB - 1:
                        nc.sync.dma_start(
                            out_d.ap()[128 * stt:128 * (stt + 1), :], osb[:])
                        del osb_live[stt]

                def kt_unit(h, sb):
                    pk = pden.tile([128, 512], F32, name="pk", tag="den")
                    for k in range(2):
                        nc.tensor.matmul(pk[:],
                                         wuk_t[:, k, 128 * h:128 * (h + 1)],
                                         latT[k][:, 512 * sb:512 * (sb + 1)],
                                         start=(k == 0), stop=(k == 1))
                    nc.vector.tensor_copy(kT[h][:, 512 * sb:512 * (sb + 1)],
                                          pk[:])

                def vt_unit(h, st4):
                    pv = pden.tile([128, 512], F32, name="pv2", tag="den")
                    for j in range(4):
                        stt = 4 * st4 + j
                        for m in range(LAT // 128):
                            nc.tensor.matmul(
                                pv[:, 128 * j:128 * (j + 1)],
                                latT[m][:, 128 * stt:128 * (stt + 1)],
                                wuv_t[:, m, 128 * h:128 * (h + 1)],
                                start=(m == 0), stop=(m == LAT // 128 - 1))
                    nc.vector.tensor_copy(vt[h][:, 512 * st4:512 * (st4 + 1)],
                                          pv[:])

                for qb in range(NB):
                    units = []
                    if qb + 1 < NB:
                        for h in range(HEADS_PER_CORE):
                            units.append((kt_unit, (h, qb + 1)))
                            units.append((vt_unit, (h, qb + 1)))
                    if qb > 0:
                        for stt in range(4 * (qb - 1), 4 * qb):
                            for ob in range(NB):
                                units.append((po_unit, (stt, ob)))
                    units = iter(units)

                    nkt = 4 * qb + 4
                    pend = []  # software pipeline (depth 2), rolls across heads

                    def finish(p):
                        hh, kt0, col0, ps_ctx_, exsum_, ex_ = p
                        # one deferred unit to fill the exp-chain latency
                        u = next(units, None)
                        if u is not None:
                            u[0](*u[1])
                        # pair-sum + running key-sum on DVE: a single
                        # all-ones denominator matmul per (h, qb)
                        if kt0 == 0:
                            nc.vector.tensor_add(exsum_[:, 0:512],
                                                 ex_[:, 0:512],
                                                 ex_[:, 512:1024])
                        else:
                            exs = tpool.tile([128, 512], F16, name="exs",
                                             tag="exs", bufs=3)
                            nc.vector.tensor_add(exs[:, col0:512],
                                                 ex_[:, col0:512],
                                                 ex_[:, 512 + col0:1024])
                            nc.vector.tensor_add(exsum_[:, col0:512],
                                                 exsum_[:, col0:512],
                                                 exs[:, col0:512])
                        for half in range(2):
                            kt = kt0 + half
                            nc.tensor.matmul(
                                ps_ctx_[:, col0:512],
                                vt[hh][:, 128 * kt:128 * (kt + 1)],
                                ex_[:, 512 * half + col0:512 * (half + 1)],
                                start=(kt0 == 0 and half == 0),
                                stop=(kt == nkt - 1))
                        if kt0 == nkt - 2:
                            # head hh's last pair retired: denominator,
                            # reciprocal, and the ctxT scale for this head
                            ps_den = pden.tile([128, 512], F32, name="ps_den",
                                               tag="den")
                            nc.tensor.matmul(ps_den[:], ones_t[:], exsum_[:],
                                             start=True, stop=True)
                            rden = tpool.tile([128, 512], F32, name="rden",
                                              tag="rden", bufs=2)
                            nc.vector.reciprocal_approx_fast(rden[:],
                                                             ps_den[:])
                            nc.vector.tensor_mul(
                                ctxT[hh][:, 512 * qb:512 * (qb + 1)],
                                ps_ctx_[:], rden[:])

                    for h in range(HEADS_PER_CORE):
                        ps_ctx = pctx.tile([128, 512], F32, name="ps_ctx",
                                           tag="ctx")
                        exsum = tpool.tile([128, 512], F16, name="exsum",
                                           tag="exsum", bufs=2)
                        for kt0 in range(0, nkt, 2):
                            pair = (kt0, kt0 + 1)
                            # valid q start (block-local) per kt; pair shares
                            # the wider (earlier) start col0 of ktA
                            djA = pair[0] - 4 * qb
                            col0 = 128 * djA if djA >= 0 else 0
                            qhi = 512 * (qb + 1)
                            ps_sc = psc.tile([128, 1024], F32, name="ps_sc",
                                             tag="sc")
                            ex = tpool.tile([128, 1024], F16, name="ex",
                                            tag="ex", bufs=4)
                            for half, kt in enumerate(pair):
                                dj = kt - 4 * qb
                                c = 128 * dj if dj >= 0 else 0
                                nc.tensor.matmul(
                                    ps_sc[:, 512 * half + c:512 * (half + 1)],
                                    kT[h][:, 128 * kt:128 * (kt + 1)],
                                    qT[h][:, 512 * qb + c:qhi],
                                    start=True, stop=True,
                                    skip_group_check=True)
                            # one wide exp for the pair (psum -> sbuf fp16);
                            # the half1 strip [512+col0, 512+col0+128) sees
                            # stale psum (bounded), gpsimd zeroes it below.
                            nc.scalar.activation(ex[:, col0:1024],
                                                 ps_sc[:, col0:1024], Exp,
                                                 scale=SCALE)
                            if djA >= 0:
                                nc.gpsimd.memset(
                                    ex[:, 512 + col0:512 + col0 + 128], 0.0)
                            for half, kt in enumerate(pair):
                                dj = kt - 4 * qb
                                if dj >= 0:
                                    c = 128 * dj
                                    nc.gpsimd.tensor_mul(
                                        ex[:, 512 * half + c:512 * half + c + 128],
                                        ex[:, 512 * half + c:512 * half + c + 128],
                                        mask_t[:])
                            pend.append((h, kt0, col0, ps_ctx, exsum, ex))
                            if len(pend) > 2:
                                finish(pend.pop(0))
                    for p in pend:
                        finish(p)

                    # any leftover deferred units, and qb3's own out-proj
                    for fn, args in units:
                        fn(*args)
                    if qb == NB - 1:
                        for stt in range(4 * qb, 4 * qb + 4):
                            for ob in range(NB):
                                po_unit(stt, ob)

    nc.compile()
    return nc


def _get_nc():
    if "nc" not in _CACHE:
        _CACHE["nc"] = _build()
    return _CACHE["nc"]


def _swz(a, chunk_rows=128):
    """[C*128, F] -> [128, C*F] chunk-major row-contiguous swizzle."""
    cr, f = a.shape
    c = cr // chunk_rows
    return np.ascontiguousarray(
        a.reshape(c, chunk_rows, f).transpose(1, 0, 2).reshape(chunk_rows,
                                                               c * f))


def _make_in_maps(x, W_query, W_DKV, W_UK, W_UV, W_out):
    mask = np.triu(np.ones((128, 128), dtype=np.float16))
    wdkv16 = _swz(W_DKV.astype(np.float16))
    xT16 = [_swz(x[b].T.astype(np.float16)) for b in range(B)]
    in_maps = []
    for c in range(NCORES):
        b = c // 4
        g = c % 4
        cols = slice(512 * g, 512 * (g + 1))
        in_maps.append({
            "xt": xT16[b],
            "wq": _swz(W_query[:, cols].astype(np.float16)),
            "wdkv": wdkv16,
            "wuk": _swz(W_UK[:, cols].astype(np.float16)),
            "wuv": _swz(W_UV[:, cols].astype(np.float16)),
            "wout": _swz(W_out[cols, :].astype(np.float16)),
            "mask": mask,
        })
    return in_maps


def run_on_device(x, W_query, W_DKV, W_UK, W_UV, W_out, **run_kwargs):
    nc = _get_nc()
    in_maps = _make_in_maps(x, W_query, W_DKV, W_UK, W_UV, W_out)
    return bass_utils.run_bass_kernel_spmd(
        nc, in_maps, core_ids=list(range(NCORES)), **run_kwargs)


def kernel(x, W_query, W_DKV, W_UK, W_UV, W_out, b_out):
    x = np.asarray(x, dtype=np.float32)
    W_query = np.asarray(W_query, dtype=np.float32)
    W_DKV = np.asarray(W_DKV, dtype=np.float32)
    W_UK = np.asarray(W_UK, dtype=np.float32)
    W_UV = np.asarray(W_UV, dtype=np.float32)
    W_out = np.asarray(W_out, dtype=np.float32)
    b_out = np.asarray(b_out, dtype=np.float32)

    res = None
    for attempt in range(3):
        try:
            res = run_on_device(x, W_query, W_DKV, W_UK, W_UV, W_out)
            break
        except Exception:
            if attempt == 2:
                raise
    out = np.empty((B, S, DOUT), dtype=np.float32)
    for b in range(B):
        acc = np.asarray(res.results[4 * b]["out"], dtype=np.float32)
        for g in range(1, 4):
            acc += np.asarray(res.results[4 * b + g]["out"], dtype=np.float32)
        out[b] = acc + b_out[None, :]
    return out
